# revision 3
# baseline (speedup 1.0000x reference)
"""Single-head causal attention (B=4, S=4096, D_IN=256, D_OUT=64) on 8 TRN2 cores.

Strategy (SPMD, one Bass program, per-core data):
  - 2 cores per batch. Per batch, the 16 query blocks of 256 rows are split by
    causal workload: member A (core%2==0) takes odd blocks {1,3,..,15} (k-chunk
    counts 4,8,..,32), member B takes even blocks {0,2,..,14} (counts 2,6,..,30,
    padded +2 junk chunks each so every core runs the identical program).
  - Program: 8 q-slots of 256 rows; slot s iterates C_s = 4(s+1) k-chunks of 128.
    The last 4 k-chunk positions of each slot are multiplied by per-core mask
    tiles (A: [1,1,M1,M2], B: [M1,M2,0,0]) which implement the causal mask and
    neutralize B's padding.
  - Layouts: host passes X^T (d_in on partitions) so projections are plain
    matmuls. Q^T/K^T [64, seq] via lhsT=W chunks (f32r); V natural [k,64] via
    lhsT=Xv^T chunks (bf16, FWL); attention S^T tile = K^T_tile^T @ Q^T (f32r),
    exp on ACT into bf16 P^T, PV accumulates lhsT=V'[k,65] (ones column fuses
    the softmax row-sum) into PSUM [65, 256]. Final PE transpose -> [q, 65],
    divide by column 64, DMA out.
"""

import numpy as np
import ml_dtypes

B, S, D_IN, D_OUT = 4, 4096, 256, 64
N_CORES = 8
QS = 256            # q rows per slot
N_SLOTS = 8         # slots per core
KC = 128            # k rows per chunk
QT = QS * N_SLOTS   # 2048 q rows per core
STRIPS = 2          # QK row-strip packing (1 or 2)

_STATE = {}


def _build_program(repeats=1):
    from contextlib import ExitStack
    import concourse.tile as tile
    from concourse import bacc, mybir

    f32 = mybir.dt.float32
    f32r = mybir.dt.float32r
    bf16 = mybir.dt.bfloat16
    Exp = mybir.ActivationFunctionType.Exp
    import concourse.bass as bass
    ts = bass.ts

    nc = bacc.Bacc("TRN2", target_bir_lowering=False, debug=False,
                   num_devices=N_CORES)

    xq = nc.dram_tensor("xq_t", [D_IN, QT], f32r, kind="ExternalInput").ap()
    xk = nc.dram_tensor("xk_t", [D_IN, S], f32r, kind="ExternalInput").ap()
    xv = nc.dram_tensor("xv_t", [D_IN, S], f32, kind="ExternalInput").ap()
    wq = nc.dram_tensor("wq", [D_IN, D_OUT], f32r, kind="ExternalInput").ap()
    wk = nc.dram_tensor("wk", [D_IN, D_OUT], f32r, kind="ExternalInput").ap()
    wv = nc.dram_tensor("wv", [D_IN, D_OUT], f32, kind="ExternalInput").ap()
    masks = nc.dram_tensor("masks", [128, 4 * QS], bf16, kind="ExternalInput").ap()
    ident = nc.dram_tensor("ident", [128, 128], f32, kind="ExternalInput").ap()
    out = nc.dram_tensor("out", [QT, D_OUT], f32, kind="ExternalOutput").ap()

    NKT = S // KC        # 32 k-tiles
    HALF = S // 2

    with tile.TileContext(nc) as tc:
        with ExitStack() as ctx:
            const = ctx.enter_context(tc.tile_pool(name="const", bufs=1))
            xin = ctx.enter_context(tc.tile_pool(name="xin", bufs=1))
            proj = ctx.enter_context(tc.tile_pool(name="proj", bufs=1))
            kt_pool = ctx.enter_context(tc.tile_pool(name="ktp", bufs=8))
            qt_pool = ctx.enter_context(tc.tile_pool(name="qtp", bufs=4))
            vp_pool = ctx.enter_context(tc.tile_pool(name="vpp", bufs=32))
            pt_pool = ctx.enter_context(tc.tile_pool(name="ptp", bufs=4))
            o_pool = ctx.enter_context(tc.tile_pool(name="op", bufs=2))
            ob_pool = ctx.enter_context(tc.tile_pool(name="obp", bufs=4))
            rc_pool = ctx.enter_context(tc.tile_pool(name="rcp", bufs=4))
            ps_a = ctx.enter_context(tc.tile_pool(name="ps_a", space="PSUM", bufs=4))
            ps_b = ctx.enter_context(tc.tile_pool(name="ps_b", space="PSUM", bufs=2))
            ps_o = ctx.enter_context(tc.tile_pool(name="ps_o", space="PSUM", bufs=2))

            # ---- constants ----
            wq_sb = const.tile([128, 2, D_OUT], f32r, tag="wq")
            nc.sync.dma_start(wq_sb[:], wq.rearrange("(c p) d -> p c d", p=128))
            wk_sb = const.tile([128, 2, D_OUT], f32r, tag="wk")
            nc.sync.dma_start(wk_sb[:], wk.rearrange("(c p) d -> p c d", p=128))
            wv_sb = const.tile([128, 2, D_OUT], bf16, tag="wv")
            nc.gpsimd.dma_start(wv_sb[:], wv.rearrange("(c p) d -> p c d", p=128))
            mask_sb = const.tile([128, 4 * QS], bf16, tag="masks")
            nc.sync.dma_start(mask_sb[:], masks[:])
            id_sb = const.tile([128, 128], f32, tag="ident")
            nc.sync.dma_start(id_sb[:], ident[:])

            # ---- transposed input loads (2 halves each for pipelining) ----
            xq_h = []
            for h in range(2):
                t = xin.tile([128, 2, QT // 2], f32r, tag=f"xq{h}")
                nc.sync.dma_start(
                    t[:], xq.rearrange("(c p) n -> p c n", p=128)[
                        :, :, h * (QT // 2):(h + 1) * (QT // 2)])
                xq_h.append(t)
            xk_h = []
            for h in range(2):
                t = xin.tile([128, 2, HALF], f32r, tag=f"xk{h}")
                nc.sync.dma_start(
                    t[:], xk.rearrange("(c p) n -> p c n", p=128)[
                        :, :, h * HALF:(h + 1) * HALF])
                xk_h.append(t)
            xv_h = []
            for h in range(2):
                t = xin.tile([128, 2, HALF], bf16, tag=f"xv{h}")
                nc.gpsimd.dma_start(
                    t[:], xv.rearrange("(c p) n -> p c n", p=128)[
                        :, :, h * HALF:(h + 1) * HALF])
                xv_h.append(t)

            kt_tiles = [None] * (S // 512)
            qt_tiles = [None] * (QT // 512)
            vp_tiles = [None] * NKT

            def k_proj(t):  # K^T chunk t: [64, 512] (+ dup rows 64:128)
                h, off = divmod(512 * t, HALF)
                ps = ps_a.tile([64, 512], f32, tag="ps_a")
                nc.tensor.matmul(ps[:], wk_sb[:, 0, :], xk_h[h][:, 0, off:off + 512],
                                 start=True, stop=False)
                nc.tensor.matmul(ps[:], wk_sb[:, 1, :], xk_h[h][:, 1, off:off + 512],
                                 start=False, stop=True)
                kt = kt_pool.tile([128, 512], f32r, tag="kt")
                nc.vector.tensor_copy(kt[0:64, :], ps[:])
                if STRIPS == 2:
                    nc.sync.dma_start(kt[64:128, :], kt[0:64, :])
                kt_tiles[t] = kt

            def q_proj(t):  # Q^T chunk t: [64, 512] (+ dup)
                h, off = divmod(512 * t, QT // 2)
                ps = ps_a.tile([64, 512], f32, tag="ps_a")
                nc.tensor.matmul(ps[:], wq_sb[:, 0, :], xq_h[h][:, 0, off:off + 512],
                                 start=True, stop=False)
                nc.tensor.matmul(ps[:], wq_sb[:, 1, :], xq_h[h][:, 1, off:off + 512],
                                 start=False, stop=True)
                qt = qt_pool.tile([128, 512], f32r, tag="qt")
                nc.vector.tensor_copy(qt[0:64, :], ps[:])
                if STRIPS == 2:
                    nc.sync.dma_start(qt[64:128, :], qt[0:64, :])
                qt_tiles[t] = qt

            def v_proj(j):  # V' tile j: [128, 65] bf16, col 64 = 1.0
                h, off = divmod(KC * j, HALF)
                ps = ps_b.tile([128, D_OUT], f32, tag="ps_b")
                nc.tensor.matmul(ps[:], xv_h[h][:, 0, off:off + KC], wv_sb[:, 0, :],
                                 start=True, stop=False)
                nc.tensor.matmul(ps[:], xv_h[h][:, 1, off:off + KC], wv_sb[:, 1, :],
                                 start=False, stop=True)
                vp = vp_pool.tile([128, D_OUT + 1], bf16, tag="vp")
                nc.vector.memset(vp[:, D_OUT:D_OUT + 1], 1.0)
                nc.vector.tensor_copy(vp[:, 0:D_OUT], ps[:])
                vp_tiles[j] = vp

            def slot(s):
                cs = 4 * (s + 1)
                qtile = qt_tiles[s // 2]
                qoff = QS * (s % 2)
                po = ps_o.tile([D_OUT + 1, QS], f32, tag="ps_o")

                def qk(j):
                    par = (j % 2) * 64 if STRIPS == 2 else 0
                    pss = ps_a.tile([128, QS], f32, tag="ps_a")
                    kt = kt_tiles[j // 4]
                    nc.tensor.matmul(
                        pss[:],
                        kt[par:par + 64, ts(j % 4, KC)],
                        qtile[par:par + 64, qoff:qoff + QS],
                        start=True, stop=True)
                    return pss

                pss_q = [None] * cs
                pss_q[0] = qk(0)
                for j in range(cs):
                    if j + 1 < cs:
                        pss_q[j + 1] = qk(j + 1)
                    pt = pt_pool.tile([128, QS], bf16, tag="pt")
                    nc.scalar.activation(pt[:], pss_q[j][:], Exp)
                    pss_q[j] = None
                    if j >= cs - 4:
                        m = j - (cs - 4)
                        nc.vector.tensor_mul(pt[:], pt[:], mask_sb[:, ts(m, QS)])
                    nc.tensor.matmul(po[:], vp_tiles[j][:], pt[:],
                                     start=(j == 0), stop=(j == cs - 1))

                osb = o_pool.tile([D_OUT + 1, QS], f32, tag="osb")
                nc.vector.tensor_copy(osb[:], po[:])
                for t2 in range(QS // 128):
                    pst = ps_b.tile([128, D_OUT + 1], f32, tag="ps_b")
                    nc.tensor.transpose(pst[:], osb[:, ts(t2, 128)],
                                        id_sb[0:D_OUT + 1, 0:D_OUT + 1])
                    rc = rc_pool.tile([128, 1], f32, tag="rc")
                    nc.vector.reciprocal(rc[:], pst[:, D_OUT:D_OUT + 1])
                    ob = ob_pool.tile([128, D_OUT], f32, tag="ob")
                    nc.vector.tensor_scalar_mul(ob[:], pst[:, 0:D_OUT], rc[:])
                    r0 = QS * s + 128 * t2
                    nc.sync.dma_start(out[r0:r0 + 128, :], ob[:])

            # emission order interleaves projections with slots so the PE
            # reaches attention work (and ACT/DVE light up) early.
            for _rep in range(repeats):
                for s in range(N_SLOTS):
                    k_proj(s)
                    if s % 2 == 0:
                        q_proj(s // 2)
                    for j in range(4 * s, 4 * s + 4):
                        v_proj(j)
                    slot(s)

    nc.compile()
    return nc


def _host_inputs(inputs):
    """Build the 8 per-core input maps."""
    xq_full = np.asarray(inputs["inputs_for_queries"], dtype=np.float32)
    xk_full = np.asarray(inputs["inputs_for_keys"], dtype=np.float32)
    xv_full = np.asarray(inputs["inputs_for_values"], dtype=np.float32)
    wq = np.asarray(inputs["wq"], dtype=np.float32) / np.sqrt(np.float32(D_OUT))
    wk = np.asarray(inputs["wk"], dtype=np.float32)
    wv = np.asarray(inputs["wv"], dtype=np.float32)

    dk = np.arange(128, dtype=np.int64)[:, None]
    dq = np.arange(QS, dtype=np.int64)[None, :]
    m1 = (dk <= dq).astype(np.float32)
    m2 = (dk + 128 <= dq).astype(np.float32)
    ones = np.ones((128, QS), np.float32)
    zeros = np.zeros((128, QS), np.float32)
    mask_a = np.concatenate([ones, ones, m1, m2], 1).astype(ml_dtypes.bfloat16)
    mask_b = np.concatenate([m1, m2, zeros, zeros], 1).astype(ml_dtypes.bfloat16)
    ident = np.eye(128, dtype=np.float32)

    in_maps = []
    for c in range(N_CORES):
        b, m = divmod(c, 2)
        blocks = [2 * s + 1 - m for s in range(N_SLOTS)]
        qsel = np.concatenate([xq_full[b, QS * i:QS * i + QS, :] for i in blocks], 0)
        in_maps.append({
            "xq_t": np.ascontiguousarray(qsel.T),
            "xk_t": np.ascontiguousarray(xk_full[b].T),
            "xv_t": np.ascontiguousarray(xv_full[b].T),
            "wq": wq, "wk": wk, "wv": wv,
            "masks": mask_b if m else mask_a,
            "ident": ident,
        })
    return in_maps


def _assemble(results):
    out = np.empty((B, S, D_OUT), dtype=np.float32)
    for c in range(N_CORES):
        b, m = divmod(c, 2)
        co = results[c]["out"]
        for s in range(N_SLOTS):
            i = 2 * s + 1 - m
            out[b, QS * i:QS * i + QS, :] = co[QS * s:QS * s + QS, :]
    return out


def _run(inputs, trace=False):
    from concourse.bass_utils import run_bass_kernel_spmd
    if "nc" not in _STATE:
        _STATE["nc"] = _build_program()
    res = run_bass_kernel_spmd(_STATE["nc"], _host_inputs(inputs),
                               list(range(N_CORES)), trace=trace)
    return _assemble(res.results), res


def kernel(**inputs):
    out, _ = _run(inputs, trace=False)
    return out
